# revision 19
# baseline (speedup 1.0000x reference)
"""Multi-head attention forward on 8 Trainium2 NeuronCores (Bass/Tile).

Problem: B=4, L=2048, D=1024, H=16 heads, DV=64.
  out = softmax((x_q Wq^T + bq)(x_k Wk^T + bk)^T / sqrt(DV)) (x_v Wv^T + bv) Wc^T + bc

Sharding (8 cores): core c handles batch b = c//2 and head-group g = c%2
(8 heads = 512 of the 1024 projection columns). Each core produces a
full-shape [L, D] partial of the output projection; the host sums the two
partials per batch and adds bc.

v2 design (vs v1 baseline at 574us):
 - All matmul operands in bf16 (fp32 PSUM accumulation). Halves input DMA,
   enables fast weight loads.
 - Scores matmuls for the two heads of a pair are issued back-to-back with
   distinct PE row-groups (K=64 at partitions 0/64) so they run concurrently
   in the PE array (row tiling).
 - Softmax exp runs on the Scalar engine (the kernel bottleneck, ~1.34us per
   [128,1024] tile); everything else is arranged to hide under it:
   projections for pair m+1 are issued as fine-grained "fillers" between the
   attention score tiles of pair m so the shared PSUM ring interleaves them,
   and the Tile run-ahead scheduler fills PE idle slots.
 - Normalization: reciprocal_approx_fast (~5x faster than iterative
   reciprocal, which cost 16 x 7.8us in v1) + DRAM-roundtrip partition
   broadcast.
 - Stage C (output projection) for the first half of rows runs as fillers
   inside the last attention slot; only the second half trails.

PSUM budget (8 banks): scores ring 2 x [128,1024] (4 banks) +
2 x AV accumulators [65,1024] (4 banks).
"""

from contextlib import ExitStack

import numpy as np
import ml_dtypes

import concourse.bacc as bacc
import concourse.mybir as mybir
from concourse.tile import TileContext
from concourse.bass_utils import run_bass_kernel_spmd

B, L, D, H = 4, 2048, 1024, 16
DV = 64
HPC = 8           # heads per core
OC = HPC * DV     # 512 projection cols per core
NCORES = 8

F32 = mybir.dt.float32
BF16 = mybir.dt.bfloat16
EXP = mybir.ActivationFunctionType.Exp

NI = D // 128    # 8 contraction tiles for projections
NM = OC // 128   # 4 head pairs
NLT = L // 128   # 16 l/k tiles
QW = 1024        # q-half width in stage B

_CACHE = {}


def _build():
    nc = bacc.Bacc("TRN2", target_bir_lowering=False, debug=False,
                   num_devices=NCORES)

    xtq = nc.dram_tensor("XTQ", [D, L], BF16, kind="ExternalInput")
    xtk = nc.dram_tensor("XTK", [D, L], BF16, kind="ExternalInput")
    xtv = nc.dram_tensor("XTV", [D, L], BF16, kind="ExternalInput")
    wqt = nc.dram_tensor("WQT", [D, OC], BF16, kind="ExternalInput")
    wkt = nc.dram_tensor("WKT", [D, OC], BF16, kind="ExternalInput")
    wvt = nc.dram_tensor("WVT", [D, OC], BF16, kind="ExternalInput")
    wct = nc.dram_tensor("WCT", [OC, D], BF16, kind="ExternalInput")
    bqd = nc.dram_tensor("BQ", [OC], F32, kind="ExternalInput")
    bkd = nc.dram_tensor("BK", [OC], F32, kind="ExternalInput")
    bvd = nc.dram_tensor("BV", [OC], F32, kind="ExternalInput")
    out = nc.dram_tensor("OUT", [L, D], F32, kind="ExternalOutput")
    attn0d = nc.dram_tensor("ATTN0", [128, L], BF16, kind="ExternalOutput")
    vext0d = nc.dram_tensor("VEXT0", [128, HPC * (DV + 1)], BF16,
                            kind="ExternalOutput")

    with TileContext(nc) as tc:
        with (
            tc.tile_pool(name="qkt", bufs=2 * NM) as qkt_pool,
            tc.tile_pool(name="vext", bufs=NLT) as vext_pool,
            tc.tile_pool(name="attnt", bufs=NM) as attnt_pool,
            tc.tile_pool(name="wc", bufs=NM) as wc_pool,
            tc.tile_pool(name="w", bufs=2 * NI) as w_pool,
            tc.tile_pool(name="wv", bufs=NI) as wv_pool,
            tc.tile_pool(name="xs", bufs=16) as xs_pool,
            tc.tile_pool(name="bias", bufs=4) as bias_pool,
            tc.tile_pool(name="ex", bufs=8) as ex_pool,
            tc.tile_pool(name="oc", bufs=2) as oc_pool,
            tc.tile_pool(name="rr", bufs=2) as rr_pool,
            tc.tile_pool(name="rb", bufs=2) as rb_pool,
            tc.tile_pool(name="ob", bufs=2) as ob_pool,
            tc.tile_pool(name="rcd", bufs=8, space="DRAM") as rcd_pool,
            tc.tile_pool(name="st", bufs=2, space="PSUM") as st_pool,
            tc.tile_pool(name="ot", bufs=2, space="PSUM") as ot_pool,
        ):
            qt = [qkt_pool.tile([128, L], BF16, tag="qkt", name=f"qt{i}")
                  for i in range(NM)]
            kt = [qkt_pool.tile([128, L], BF16, tag="qkt", name=f"kt{i}")
                  for i in range(NM)]
            vext = [vext_pool.tile([128, HPC, DV + 1], BF16, tag="vext",
                                   name=f"vext{i}")
                    for i in range(NLT)]
            attnt = [attnt_pool.tile([128, L], BF16, tag="attnt",
                                     name=f"attnt{m}")
                     for m in range(NM)]

            # ---- biases + weights ----
            qb_tile = bias_pool.tile([128, NM], F32, tag="b1", name="bqt",
                                     bufs=2)
            nc.sync.dma_start(
                out=qb_tile, in_=bqd[:].rearrange("(m p) -> p m", p=128))
            kb_tile = bias_pool.tile([128, NM], F32, tag="b1", name="bkt",
                                     bufs=2)
            nc.sync.dma_start(
                out=kb_tile, in_=bkd[:].rearrange("(m p) -> p m", p=128))
            qbias = [qb_tile[:, mm_:mm_ + 1] for mm_ in range(NM)]
            kbias = [kb_tile[:, mm_:mm_ + 1] for mm_ in range(NM)]
            vbias = bias_pool.tile([128, OC], F32, tag="bv", bufs=1)
            nc.sync.dma_start(
                out=vbias, in_=bvd[:].unsqueeze(0).to_broadcast((128, OC)))
            onesf = bias_pool.tile([128, HPC], BF16, tag="ones", bufs=1)
            nc.vector.memset(onesf, 1.0)

            wq_tiles, wk_tiles, wv_tiles, wc_tiles = [], [], [], []
            for i in range(NI):
                t = w_pool.tile([128, OC], BF16, tag="wq", name="wqt_t")
                nc.sync.dma_start(out=t, in_=wqt[i * 128:(i + 1) * 128, :])
                wq_tiles.append(t)
            for i in range(NI):
                t = w_pool.tile([128, OC], BF16, tag="wk", name="wkt_t")
                nc.sync.dma_start(out=t, in_=wkt[i * 128:(i + 1) * 128, :])
                wk_tiles.append(t)
            for i in range(NI):
                t = wv_pool.tile([128, OC], BF16, tag="wv", name="wvt_t")
                nc.gpsimd.dma_start(out=t, in_=wvt[i * 128:(i + 1) * 128, :])
                wv_tiles.append(t)
            def load_wc():
                for dt in range(NM):
                    t = wc_pool.tile([128, D], BF16, tag="wc", name="wct_t")
                    nc.gpsimd.dma_start(
                        out=t, in_=wct[dt * 128:(dt + 1) * 128, :])
                    wc_tiles.append(t)

            # ---- projection helpers ----
            def qk_proj_unit(m, w_tiles, xsrc, dst, bias, lcp, dmae):
                """One PSUM-ring unit: projection of pair m for l-cols
                [lcp*1024, (lcp+1)*1024). Bias-add lands per 512-col chunk
                so consumers unblock as early as possible."""
                ps = st_pool.tile([128, QW], F32, tag="st", name="psp")
                x_tiles = []
                for i in range(NI):
                    t = xs_pool.tile([128, QW], BF16, tag="xs", name="xp")
                    dmae.dma_start(
                        out=t,
                        in_=xsrc[i * 128:(i + 1) * 128,
                                 lcp * QW:(lcp + 1) * QW])
                    x_tiles.append(t)
                for sub in range(2):
                    for i in range(NI):
                        nc.tensor.matmul(
                            ps[:, sub * 512:(sub + 1) * 512],
                            lhsT=w_tiles[i][:, m * 128:(m + 1) * 128],
                            rhs=x_tiles[i][:, sub * 512:(sub + 1) * 512],
                            start=(i == 0), stop=(i == NI - 1))
                    nc.vector.tensor_add(
                        dst[m][:, lcp * QW + sub * 512:
                               lcp * QW + (sub + 1) * 512],
                        ps[:, sub * 512:(sub + 1) * 512],
                        bias[m].to_broadcast((128, 512)))

            def qk_proj_fillers(m):
                return [
                    lambda m=m, lcp=lcp, w=w, x=x, d=d, b=b, e=e: qk_proj_unit(
                        m, w, x, d, b, lcp, e)
                    for (w, x, d, b, e) in (
                        (wk_tiles, xtk, kt, kbias, nc.sync),
                        (wq_tiles, xtq, qt, qbias, nc.gpsimd))
                    for lcp in range(2)
                ]

            def v_proj_chunk(lc):
                """V projection for the 4 l-subtiles of l-chunk lc."""
                x_tiles = []
                for i in range(NI):
                    t = xs_pool.tile([128, 512], BF16, tag="xv", name="xv",
                                     bufs=8)
                    nc.gpsimd.dma_start(
                        out=t,
                        in_=xtv[i * 128:(i + 1) * 128,
                                lc * 512:(lc + 1) * 512])
                    x_tiles.append(t)
                for lsp in range(2):
                    ps = st_pool.tile([128, QW], F32, tag="st", name="psv")
                    for sub in range(2):
                        ls = lsp * 2 + sub
                        for i in range(NI):
                            nc.tensor.matmul(
                                ps[:, sub * 512:(sub + 1) * 512],
                                lhsT=x_tiles[i][:, ls * 128:(ls + 1) * 128],
                                rhs=wv_tiles[i],
                                start=(i == 0), stop=(i == NI - 1))
                    for sub in range(2):
                        lt = lc * 4 + lsp * 2 + sub
                        nc.vector.tensor_add(
                            vext[lt][:, :, 0:DV],
                            ps[:, sub * 512:(sub + 1) * 512].rearrange(
                                "p (h d) -> p h d", h=HPC),
                            vbias.rearrange("p (h d) -> p h d", h=HPC))
                        nc.vector.tensor_copy(vext[lt][:, :, DV], onesf)

            # ---- stage B: attention for (pair m, q-half qc) ----
            def attention(m, qc, fillers):
                """fillers: list of closures issued spread through the
                k-loop so shared-PSUM-ring neighbors interleave."""
                fill_at = {}
                if fillers:
                    step = (2 * NLT) // len(fillers)
                    for fi in range(len(fillers)):
                        fill_at.setdefault(2 + fi * step, []).append(
                            fillers[fi])
                ot = [ot_pool.tile([DV + 1, QW], F32, tag="ot",
                                   name=f"ot{h2}")
                      for h2 in range(2)]
                exs = {}

                def scores_exp(h2, k):
                    off = h2 * DV
                    st = st_pool.tile([128, QW], F32, tag="st", name="st")
                    for j in range(2):
                        nc.tensor.matmul(
                            st[:, j * 512:(j + 1) * 512],
                            lhsT=kt[m][off:off + DV,
                                       k * 128:(k + 1) * 128],
                            rhs=qt[m][off:off + DV,
                                      qc * QW + j * 512:
                                      qc * QW + (j + 1) * 512],
                            start=True, stop=True)
                    ex = ex_pool.tile([128, QW], BF16, tag="ex", name="ex")
                    nc.scalar.activation(out=ex, in_=st, func=EXP,
                                         scale=0.125)
                    exs[(h2, k)] = ex

                def av(h2, k):
                    ex = exs.pop((h2, k))
                    hg = m * 2 + h2
                    for j in range(2):
                        nc.tensor.matmul(
                            ot[h2][:, j * 512:(j + 1) * 512],
                            lhsT=vext[k][:, hg, :],
                            rhs=ex[:, j * 512:(j + 1) * 512],
                            start=(k == 0), stop=(k == NLT - 1))

                ustep = 0
                for k in range(NLT):
                    for h2 in range(2):
                        for f in fill_at.get(ustep, ()):
                            f()
                        scores_exp(h2, k)
                        ustep += 1
                    if k >= 2:
                        av(0, k - 2)
                        av(1, k - 2)
                for k in (NLT - 2, NLT - 1):
                    av(0, k)
                    av(1, k)
                finishers = []
                for h2 in range(2):
                    off = h2 * DV
                    # Stage ot out of PSUM immediately so the next slot's
                    # AV accumulation is not blocked by the (long-latency)
                    # reciprocal/broadcast chain, which runs off SBUF:
                    # den row -> DRAM -> [128, 8] layout -> reciprocal
                    # (8 elem/lane) -> DRAM -> partition-broadcast DMA.
                    # The final multiply is deferred into the next slot so
                    # the in-order DVE queue is not blocked waiting on the
                    # DMA chain.
                    ocs = oc_pool.tile([DV + 1, QW], F32, tag="oc",
                                       name="ocs")
                    nc.vector.tensor_copy(ocs, ot[h2])
                    dend = rcd_pool.tile([QW], F32, tag="rcd", name="dend")
                    nc.sync.dma_start(out=dend[:].unsqueeze(0),
                                      in_=ocs[DV:DV + 1, :])
                    dt128 = rr_pool.tile([128, QW // 128], F32, tag="dt",
                                         name="dt128")
                    nc.sync.dma_start(
                        out=dt128,
                        in_=dend[:].rearrange("(b p) -> p b", p=128))
                    rt128 = rr_pool.tile([128, QW // 128], F32, tag="rt",
                                         name="rt128")
                    nc.vector.reciprocal(rt128, dt128)
                    rdend = rcd_pool.tile([QW], F32, tag="rcd", name="rdend")
                    nc.sync.dma_start(
                        out=rdend[:].rearrange("(b p) -> p b", p=128),
                        in_=rt128)
                    rb = rb_pool.tile([DV, QW], F32, tag="rb", name="rb")
                    nc.gpsimd.dma_start(
                        out=rb,
                        in_=rdend[:].unsqueeze(0).to_broadcast((DV, QW)))

                    def finish(off=off, ocs=ocs, rb=rb):
                        nc.vector.tensor_mul(
                            attnt[m][off:off + DV, qc * QW:(qc + 1) * QW],
                            ocs[0:DV, :],
                            rb)
                    finishers.append(finish)
                return finishers

            # ---- stage C: output projection for one l-tile ----
            def stage_c_unit(lt, on_act):
                ps = st_pool.tile([128, QW], F32, tag="st", name="pso")
                for nck in range(2):
                    for dt in range(NM):
                        nc.tensor.matmul(
                            ps[:, nck * 512:(nck + 1) * 512],
                            lhsT=attnt[dt][:, lt * 128:(lt + 1) * 128],
                            rhs=wc_tiles[dt][:, nck * 512:(nck + 1) * 512],
                            start=(dt == 0), stop=(dt == NM - 1))
                ob = ob_pool.tile([128, QW], F32, tag="ob", name="ob")
                if on_act:
                    nc.scalar.copy(ob, ps)
                else:
                    nc.vector.tensor_copy(ob, ps)
                nc.sync.dma_start(
                    out=out[lt * 128:(lt + 1) * 128, :], in_=ob)

            # ---- issue order (= scheduler priority order) ----
            # QK(0) + V(lc0) first so the first exp unblocks ASAP; the rest
            # of the V projection and the projections for pair m+1 run as
            # fillers inside the attention slots (and stage C rows 0..7
            # inside the last slot).
            qk_proj_unit(0, wk_tiles, xtk, kt, kbias, 0, nc.sync)
            qk_proj_unit(0, wq_tiles, xtq, qt, qbias, 0, nc.gpsimd)
            qk_proj_unit(0, wk_tiles, xtk, kt, kbias, 1, nc.sync)
            v_proj_chunk(0)
            qk_proj_unit(0, wq_tiles, xtq, qt, qbias, 1, nc.gpsimd)

            fprev = []
            for m in range(NM):
                if m == 1:
                    load_wc()
                nxt = qk_proj_fillers(m + 1) if m + 1 < NM else []
                if m == 0:
                    fprev = attention(0, 0, fprev + [
                        (lambda lc=lc: v_proj_chunk(lc)) for lc in (1, 2, 3)
                    ])
                    fprev = attention(0, 1, fprev + nxt)
                elif m + 1 < NM:
                    fprev = attention(m, 0, fprev + nxt[0:2])
                    fprev = attention(m, 1, fprev + nxt[2:4])
                else:
                    fprev = attention(m, 0, fprev)
                    fprev = attention(m, 1, fprev + [
                        (lambda lt=lt: stage_c_unit(lt, False))
                        for lt in range(NLT // 2)
                    ])
            for f in fprev:
                f()
            for lt in range(NLT // 2, NLT):
                stage_c_unit(lt, True)
            nc.sync.dma_start(out=attn0d[:, :], in_=attnt[0])
            nc.sync.dma_start(
                out=vext0d[:, :],
                in_=vext[0].rearrange("p h d -> p (h d)"))

    nc.compile()
    return nc


def _get_nc():
    if "nc" not in _CACHE:
        _CACHE["nc"] = _build()
    return _CACHE["nc"]


def kernel(query, key, value, Wq, bq, Wk, bk, Wv, bv, Wc, bc, **_unused):
    BF = ml_dtypes.bfloat16
    query = np.asarray(query, np.float32)
    key = np.asarray(key, np.float32)
    value = np.asarray(value, np.float32)
    Wq = np.asarray(Wq, np.float32)
    Wk = np.asarray(Wk, np.float32)
    Wv = np.asarray(Wv, np.float32)
    Wc = np.asarray(Wc, np.float32)
    bq = np.asarray(bq, np.float32)
    bk = np.asarray(bk, np.float32)
    bv = np.asarray(bv, np.float32)
    bc = np.asarray(bc, np.float32)

    nc = _get_nc()

    xtq = [np.ascontiguousarray(query[b].T.astype(BF)) for b in range(B)]
    xtk = [np.ascontiguousarray(key[b].T.astype(BF)) for b in range(B)]
    xtv = [np.ascontiguousarray(value[b].T.astype(BF)) for b in range(B)]
    wqt_g = [np.ascontiguousarray(Wq[g * OC:(g + 1) * OC, :].T.astype(BF))
             for g in range(2)]
    wkt_g = [np.ascontiguousarray(Wk[g * OC:(g + 1) * OC, :].T.astype(BF))
             for g in range(2)]
    wvt_g = [np.ascontiguousarray(Wv[g * OC:(g + 1) * OC, :].T.astype(BF))
             for g in range(2)]
    wct_g = [np.ascontiguousarray(Wc[:, g * OC:(g + 1) * OC].T.astype(BF))
             for g in range(2)]

    in_maps = []
    for c in range(NCORES):
        b, g = c // 2, c % 2
        in_maps.append({
            "XTQ": xtq[b], "XTK": xtk[b], "XTV": xtv[b],
            "WQT": wqt_g[g], "WKT": wkt_g[g], "WVT": wvt_g[g],
            "WCT": wct_g[g],
            "BQ": np.ascontiguousarray(bq[g * OC:(g + 1) * OC]),
            "BK": np.ascontiguousarray(bk[g * OC:(g + 1) * OC]),
            "BV": np.ascontiguousarray(bv[g * OC:(g + 1) * OC]),
        })

    res = run_bass_kernel_spmd(nc, in_maps, core_ids=list(range(NCORES)),
                               **_CACHE.get("run_kwargs", {}))
    _CACHE["last_results"] = res

    outp = np.empty((B, L, D), np.float32)
    for b in range(B):
        outp[b] = res.results[2 * b]["OUT"] + res.results[2 * b + 1]["OUT"]
    outp += bc
    return outp


# revision 20
# speedup vs baseline: 1.3463x; 1.3463x over previous
"""Multi-head attention forward on 8 Trainium2 NeuronCores (Bass/Tile).

Problem: B=4, L=2048, D=1024, H=16 heads, DV=64.
  out = softmax((x_q Wq^T + bq)(x_k Wk^T + bk)^T / sqrt(DV)) (x_v Wv^T + bv) Wc^T + bc

Sharding (8 cores): core c handles batch b = c//2 and head-group g = c%2
(8 heads = 512 of the 1024 projection columns). Each core produces a
full-shape [L, D] partial of the output projection (contraction over its
512 attention-output dims); the host sums the two partials per batch and
adds bc.

Per-core pipeline (all matmuls fp32r = full-rate fp32 on the PE):
  A. V projection -> natural layout [2048, 8*65] with a ones column per
     head (rowsum trick), then Q/K projections for head-pair 0
     (QT/KT [128, 2048] per pair).
  B. Per head-pair m: attention for its 2 heads. Inner loop per
     (head, q-half): 16 k-tiles; scores^T [k=128, q=1024] in PSUM ->
     ACT exp (scale=1/8, fused) -> SBUF fp32r; AV accumulates [65, 1024]
     in PSUM (ones row = softmax denominator). ACT exp overlaps the PE stream; AV lags 2 k-tiles so all
     semaphore waits are pre-satisfied and the PE stays dense and
     HAM-warm.  Softmax denominator is applied post-AV on
     [64, 1024] tiles (reciprocal + DRAM-roundtrip partition broadcast).
  C. Output projection tail: out[l, n] accumulated over the 4 d-tiles.
"""

from contextlib import ExitStack

import numpy as np

import concourse.bacc as bacc
import concourse.mybir as mybir
from concourse.tile import TileContext
from concourse.bass_utils import run_bass_kernel_spmd

B, L, D, H = 4, 2048, 1024, 16
DV = 64
HPC = 8           # heads per core
OC = HPC * DV     # 512 projection cols per core
NCORES = 8

F32 = mybir.dt.float32
F32R = mybir.dt.float32r
EXP = mybir.ActivationFunctionType.Exp

NI = D // 128    # 8 contraction tiles for projections
NM = OC // 128   # 4 head pairs
NLT = L // 128   # 16 l/k tiles
QW = 1024        # q-half width in stage B

_CACHE = {}


def _build():
    nc = bacc.Bacc("TRN2", target_bir_lowering=False, debug=False,
                   num_devices=NCORES)

    xtq = nc.dram_tensor("XTQ", [D, L], F32R, kind="ExternalInput")
    xtk = nc.dram_tensor("XTK", [D, L], F32R, kind="ExternalInput")
    xtv = nc.dram_tensor("XTV", [D, L], F32R, kind="ExternalInput")
    wqt = nc.dram_tensor("WQT", [D, OC], F32R, kind="ExternalInput")
    wkt = nc.dram_tensor("WKT", [D, OC], F32R, kind="ExternalInput")
    wvt = nc.dram_tensor("WVT", [D, OC], F32R, kind="ExternalInput")
    wct = nc.dram_tensor("WCT", [OC, D], F32R, kind="ExternalInput")
    bqd = nc.dram_tensor("BQ", [OC], F32, kind="ExternalInput")
    bkd = nc.dram_tensor("BK", [OC], F32, kind="ExternalInput")
    bvd = nc.dram_tensor("BV", [OC], F32, kind="ExternalInput")
    out = nc.dram_tensor("OUT", [L, D], F32, kind="ExternalOutput")

    with TileContext(nc) as tc:
        with (
            tc.tile_pool(name="qkt", bufs=2 * NM) as qkt_pool,
            tc.tile_pool(name="vext", bufs=NLT) as vext_pool,
            tc.tile_pool(name="rcd", bufs=2, space="DRAM") as rcd_pool,
            tc.tile_pool(name="st", bufs=3, space="PSUM") as st_pool,
            tc.tile_pool(name="ot", bufs=1, space="PSUM") as ot_pool,
        ):
            qt = [qkt_pool.tile([128, L], F32R, tag="qkt", name=f"qt{i}")
                  for i in range(NM)]
            kt = [qkt_pool.tile([128, L], F32R, tag="qkt", name=f"kt{i}")
                  for i in range(NM)]
            vext = [vext_pool.tile([128, HPC, DV + 1], F32R, name=f"vext{i}",
                                   tag="vext")
                    for i in range(NLT)]

            # --- stage-A pools (weights, biases, x chunks) ---
            astack = ExitStack()
            xt_pool = astack.enter_context(tc.tile_pool(name="xt", bufs=16))
            w_pool = astack.enter_context(tc.tile_pool(name="w", bufs=NI))
            bias_pool = astack.enter_context(
                tc.tile_pool(name="bias", bufs=2 * NM))
            qb_tile = bias_pool.tile([128, NM], F32, tag="b1", name="bqt",
                                     bufs=2)
            nc.sync.dma_start(
                out=qb_tile, in_=bqd[:].rearrange("(m p) -> p m", p=128))
            kb_tile = bias_pool.tile([128, NM], F32, tag="b1", name="bkt",
                                     bufs=2)
            nc.sync.dma_start(
                out=kb_tile, in_=bkd[:].rearrange("(m p) -> p m", p=128))
            qbias = [qb_tile[:, mm_:mm_ + 1] for mm_ in range(NM)]
            kbias = [kb_tile[:, mm_:mm_ + 1] for mm_ in range(NM)]
            vbias = bias_pool.tile([128, OC], F32, tag="bv", bufs=1)
            nc.sync.dma_start(
                out=vbias, in_=bvd[:].unsqueeze(0).to_broadcast((128, OC)))
            onesf = bias_pool.tile([128, HPC], F32, tag="ones", bufs=1)
            nc.vector.memset(onesf, 1.0)

            # ---- stage A1: V projection ----
            with tc.tile_pool(name="wv", bufs=NI) as wv_pool:
                wv_tiles = []
                xv_list = []
                for i in range(NI):
                    t = wv_pool.tile([128, OC], F32R, tag="wv", name="wvt_t")
                    if i == 0:
                        nc.sync.dma_start(out=t[:, 0:256], in_=wvt[0:128, 0:256])
                        nc.sync.dma_start(out=t[:, 256:512],
                                          in_=wvt[0:128, 256:512])
                    else:
                        nc.sync.dma_start(
                            out=t, in_=wvt[i * 128:(i + 1) * 128, :])
                    wv_tiles.append(t)
                    t = xt_pool.tile([128, 512], F32R, tag="xt", name="xv")
                    if i == 0:
                        nc.gpsimd.dma_start(out=t[:, 0:256], in_=xtv[0:128, 0:256])
                        nc.gpsimd.dma_start(out=t[:, 256:512],
                                            in_=xtv[0:128, 256:512])
                    else:
                        nc.gpsimd.dma_start(
                            out=t, in_=xtv[i * 128:(i + 1) * 128, 0:512])
                    xv_list.append(t)
                xv_chunks = {0: xv_list}
                wq_tiles, wk_tiles = [], []
                for lc in range(4):
                    if lc in xv_chunks:
                        x_tiles = xv_chunks[lc]
                    else:
                        x_tiles = []
                        for i in range(NI):
                            t = xt_pool.tile([128, 512], F32R, tag="xt",
                                             name="xv")
                            nc.sync.dma_start(
                                out=t,
                                in_=xtv[i * 128:(i + 1) * 128,
                                        lc * 512:(lc + 1) * 512])
                            x_tiles.append(t)
                    if lc == 1:
                        for i in range(NI):
                            t = w_pool.tile([128, OC], F32R, tag="wq",
                                            name="wqt_t")
                            nc.sync.dma_start(
                                out=t, in_=wqt[i * 128:(i + 1) * 128, :])
                            wq_tiles.append(t)
                    elif lc == 2:
                        for i in range(NI):
                            t = w_pool.tile([128, OC], F32R, tag="wk",
                                            name="wkt_t")
                            nc.sync.dma_start(
                                out=t, in_=wkt[i * 128:(i + 1) * 128, :])
                            wk_tiles.append(t)
                    for lsp in range(2):   # two l-subtiles share one psum tile
                        ps = st_pool.tile([128, QW], F32, tag="st", name="psv")
                        for sub in range(2):
                            ls = lsp * 2 + sub
                            for i in range(NI):
                                nc.tensor.matmul(
                                    ps[:, sub * 512:(sub + 1) * 512],
                                    lhsT=x_tiles[i][:, ls * 128:(ls + 1) * 128],
                                    rhs=wv_tiles[i],
                                    start=(i == 0), stop=(i == NI - 1))
                        for sub in range(2):
                            lt = lc * 4 + lsp * 2 + sub
                            nc.vector.tensor_add(
                                vext[lt][:, :, 0:DV],
                                ps[:, sub * 512:(sub + 1) * 512].rearrange(
                                    "p (h d) -> p h d", h=HPC),
                                vbias.rearrange("p (h d) -> p h d", h=HPC))
                            nc.vector.tensor_copy(vext[lt][:, :, DV], onesf)

            # ---- stage A2: Q/K projections, all pairs, one pass over xT ----
            for w_tiles, xsrc, dst, biases in (
                (wq_tiles, xtq, qt, qbias),
                (wk_tiles, xtk, kt, kbias),
            ):
                for lc in range(4):
                    x_tiles = []
                    for i in range(NI):
                        t = xt_pool.tile([128, 512], F32R, tag="xt", name="xp")
                        nc.sync.dma_start(
                            out=t,
                            in_=xsrc[i * 128:(i + 1) * 128,
                                     lc * 512:(lc + 1) * 512])
                        x_tiles.append(t)
                    for mp in range(2):
                        ps = st_pool.tile([128, QW], F32, tag="st", name="psp")
                        for sub in range(2):
                            m = mp * 2 + sub
                            for i in range(NI):
                                nc.tensor.matmul(
                                    ps[:, sub * 512:(sub + 1) * 512],
                                    lhsT=w_tiles[i][:, m * 128:(m + 1) * 128],
                                    rhs=x_tiles[i],
                                    start=(i == 0), stop=(i == NI - 1))
                        for sub in range(2):
                            m = mp * 2 + sub
                            nc.vector.tensor_add(
                                dst[m][:, lc * 512:(lc + 1) * 512],
                                ps[:, sub * 512:(sub + 1) * 512],
                                biases[m].to_broadcast((128, 512)))

            astack.close()

            # ---- stage B: attention per pair ----
            attnt = {}
            bstack = ExitStack()
            attnt_pool = bstack.enter_context(tc.tile_pool(name="attnt",
                                                           bufs=NM))
            ex_pool = bstack.enter_context(tc.tile_pool(name="ex", bufs=3))
            oc_pool = bstack.enter_context(tc.tile_pool(name="oc", bufs=2))
            rr_pool = bstack.enter_context(tc.tile_pool(name="rr", bufs=1))
            wc_pool = bstack.enter_context(tc.tile_pool(name="wc", bufs=NM))
            ob_pool = bstack.enter_context(tc.tile_pool(name="ob", bufs=3))
            wc_tiles = []
            for dt in range(NM):
                t = wc_pool.tile([128, D], F32R, tag="wc", name="wct_t")
                nc.sync.dma_start(out=t, in_=wct[dt * 128:(dt + 1) * 128, :])
                wc_tiles.append(t)
            for m in range(NM):
                attnt[m] = attnt_pool.tile([128, L], F32R, tag="attnt",
                                           name=f"attnt{m}")
                for h2 in range(2):
                    h = m * 2 + h2
                    off = h2 * DV
                    for qc in range(2):
                        ot = ot_pool.tile([DV + 1, QW], F32, tag="ot",
                                          name="ot")
                        sts = {}
                        exs = {}

                        def st_step(k):
                            st = st_pool.tile([128, QW], F32, tag="st",
                                              name="st")
                            for j in range(2):
                                nc.tensor.matmul(
                                    st[:, j * 512:(j + 1) * 512],
                                    lhsT=kt[m][off:off + DV,
                                               k * 128:(k + 1) * 128],
                                    rhs=qt[m][off:off + DV,
                                              qc * QW + j * 512:
                                              qc * QW + (j + 1) * 512],
                                    start=True, stop=True)
                            ex = ex_pool.tile([128, QW], F32R, tag="ex",
                                              name="ex")
                            nc.scalar.activation(
                                out=ex, in_=st, func=EXP, scale=0.125)
                            exs[k] = ex

                        def av_step(k):
                            ex = exs.pop(k)
                            for j in range(2):
                                nc.tensor.matmul(
                                    ot[:, j * 512:(j + 1) * 512],
                                    lhsT=vext[k][:, h, :],
                                    rhs=ex[:, j * 512:(j + 1) * 512],
                                    start=(k == 0), stop=(k == NLT - 1))

                        for k in range(NLT):
                            st_step(k)
                            if k >= 2:
                                av_step(k - 2)
                        av_step(NLT - 2)
                        av_step(NLT - 1)

                        # normalization: stage ot out of PSUM, then scale
                        ocs = oc_pool.tile([DV + 1, QW], F32, tag="oc",
                                           name="ocs")
                        nc.vector.tensor_copy(ocs, ot)
                        rc = rr_pool.tile([1, QW], F32, tag="rc", name="rc")
                        nc.vector.reciprocal(rc, ocs[DV:DV + 1, :])
                        rcd = rcd_pool.tile([QW], F32, tag="rcd", name="rcd")
                        nc.sync.dma_start(out=rcd[:].unsqueeze(0), in_=rc)
                        rb = rr_pool.tile([DV, QW], F32, tag="rb", name="rb")
                        nc.gpsimd.dma_start(
                            out=rb,
                            in_=rcd[:].unsqueeze(0).to_broadcast((DV, QW)))
                        nc.vector.tensor_mul(
                            attnt[m][off:off + DV, qc * QW:(qc + 1) * QW],
                            ocs[0:DV, :],
                            rb)

            # ---- stage C: output projection ----
            for lt in range(NLT):
                ps = st_pool.tile([128, QW], F32, tag="st", name="pso")
                for nck in range(2):
                    for dt in range(NM):
                        nc.tensor.matmul(
                            ps[:, nck * 512:(nck + 1) * 512],
                            lhsT=attnt[dt][:, lt * 128:(lt + 1) * 128],
                            rhs=wc_tiles[dt][:, nck * 512:(nck + 1) * 512],
                            start=(dt == 0), stop=(dt == NM - 1))
                ob = ob_pool.tile([128, QW], F32, tag="ob", name="ob")
                nc.vector.tensor_copy(ob, ps)
                nc.sync.dma_start(
                    out=out[lt * 128:(lt + 1) * 128, :], in_=ob)
            bstack.close()

    nc.compile()
    return nc


def _get_nc():
    if "nc" not in _CACHE:
        _CACHE["nc"] = _build()
    return _CACHE["nc"]


def kernel(query, key, value, Wq, bq, Wk, bk, Wv, bv, Wc, bc, **_unused):
    query = np.asarray(query, np.float32)
    key = np.asarray(key, np.float32)
    value = np.asarray(value, np.float32)
    Wq = np.asarray(Wq, np.float32)
    Wk = np.asarray(Wk, np.float32)
    Wv = np.asarray(Wv, np.float32)
    Wc = np.asarray(Wc, np.float32)
    bq = np.asarray(bq, np.float32)
    bk = np.asarray(bk, np.float32)
    bv = np.asarray(bv, np.float32)
    bc = np.asarray(bc, np.float32)

    nc = _get_nc()

    xtq = [np.ascontiguousarray(query[b].T) for b in range(B)]
    xtk = [np.ascontiguousarray(key[b].T) for b in range(B)]
    xtv = [np.ascontiguousarray(value[b].T) for b in range(B)]
    wqt_g = [np.ascontiguousarray(Wq[g * OC:(g + 1) * OC, :].T) for g in range(2)]
    wkt_g = [np.ascontiguousarray(Wk[g * OC:(g + 1) * OC, :].T) for g in range(2)]
    wvt_g = [np.ascontiguousarray(Wv[g * OC:(g + 1) * OC, :].T) for g in range(2)]
    wct_g = [np.ascontiguousarray(Wc[:, g * OC:(g + 1) * OC].T) for g in range(2)]

    in_maps = []
    for c in range(NCORES):
        b, g = c // 2, c % 2
        in_maps.append({
            "XTQ": xtq[b], "XTK": xtk[b], "XTV": xtv[b],
            "WQT": wqt_g[g], "WKT": wkt_g[g], "WVT": wvt_g[g],
            "WCT": wct_g[g],
            "BQ": np.ascontiguousarray(bq[g * OC:(g + 1) * OC]),
            "BK": np.ascontiguousarray(bk[g * OC:(g + 1) * OC]),
            "BV": np.ascontiguousarray(bv[g * OC:(g + 1) * OC]),
        })

    res = run_bass_kernel_spmd(nc, in_maps, core_ids=list(range(NCORES)),
                               **_CACHE.get("run_kwargs", {}))
    _CACHE["last_results"] = res

    outp = np.empty((B, L, D), np.float32)
    for b in range(B):
        outp[b] = res.results[2 * b]["OUT"] + res.results[2 * b + 1]["OUT"]
    outp += bc
    return outp



# revision 21
# speedup vs baseline: 1.3717x; 1.0188x over previous
"""Multi-head attention forward on 8 Trainium2 NeuronCores (Bass/Tile).

Problem: B=4, L=2048, D=1024, H=16 heads, DV=64.
  out = softmax((x_q Wq^T + bq)(x_k Wk^T + bk)^T / sqrt(DV)) (x_v Wv^T + bv) Wc^T + bc

Sharding (8 cores): core c handles batch b = c//2 and head-group g = c%2
(8 heads = 512 of the 1024 projection columns). Each core produces a
full-shape [L, D] partial of the output projection (contraction over its
512 attention-output dims); the host sums the two partials per batch and
adds bc.

Per-core pipeline (all matmuls fp32r = full-rate fp32 on the PE):
  A. V projection -> natural layout [2048, 8*65] with a ones column per
     head (rowsum trick), then Q/K projections for head-pair 0
     (QT/KT [128, 2048] per pair).
  B. Per head-pair m: attention for its 2 heads. Inner loop per
     (head, q-half): 16 k-tiles; scores^T [k=128, q=1024] in PSUM ->
     ACT exp (scale=1/8, fused) -> SBUF fp32r; AV accumulates [65, 1024]
     in PSUM (ones row = softmax denominator). ACT exp overlaps the PE stream; AV lags 2 k-tiles so all
     semaphore waits are pre-satisfied and the PE stays dense and
     HAM-warm.  Softmax denominator is applied post-AV on
     [64, 1024] tiles (reciprocal + DRAM-roundtrip partition broadcast).
  C. Output projection tail: out[l, n] accumulated over the 4 d-tiles.
"""

from contextlib import ExitStack

import numpy as np

import concourse.bacc as bacc
import concourse.mybir as mybir
from concourse.tile import TileContext
from concourse.bass_utils import run_bass_kernel_spmd

B, L, D, H = 4, 2048, 1024, 16
DV = 64
HPC = 8           # heads per core
OC = HPC * DV     # 512 projection cols per core
NCORES = 8

F32 = mybir.dt.float32
F32R = mybir.dt.float32r
BF16 = mybir.dt.bfloat16
EXP = mybir.ActivationFunctionType.Exp

NI = D // 128    # 8 contraction tiles for projections
NM = OC // 128   # 4 head pairs
NLT = L // 128   # 16 l/k tiles
QW = 1024        # q-half width in stage B

_CACHE = {}


def _build():
    nc = bacc.Bacc("TRN2", target_bir_lowering=False, debug=False,
                   num_devices=NCORES)

    xtq = nc.dram_tensor("XTQ", [D, L], F32R, kind="ExternalInput")
    xtk = nc.dram_tensor("XTK", [D, L], F32R, kind="ExternalInput")
    xtv = nc.dram_tensor("XTV", [D, L], F32R, kind="ExternalInput")
    wqt = nc.dram_tensor("WQT", [D, OC], F32R, kind="ExternalInput")
    wkt = nc.dram_tensor("WKT", [D, OC], F32R, kind="ExternalInput")
    wvt = nc.dram_tensor("WVT", [D, OC], F32R, kind="ExternalInput")
    wct = nc.dram_tensor("WCT", [OC, D], F32R, kind="ExternalInput")
    bqd = nc.dram_tensor("BQ", [OC], F32, kind="ExternalInput")
    bkd = nc.dram_tensor("BK", [OC], F32, kind="ExternalInput")
    bvd = nc.dram_tensor("BV", [OC], F32, kind="ExternalInput")
    out = nc.dram_tensor("OUT", [L, D], F32, kind="ExternalOutput")

    with TileContext(nc) as tc:
        with (
            tc.tile_pool(name="qkt", bufs=2 * NM) as qkt_pool,
            tc.tile_pool(name="vext", bufs=NLT) as vext_pool,
            tc.tile_pool(name="rcd", bufs=2, space="DRAM") as rcd_pool,
            tc.tile_pool(name="st", bufs=3, space="PSUM") as st_pool,
            tc.tile_pool(name="ot", bufs=1, space="PSUM") as ot_pool,
        ):
            qt = [qkt_pool.tile([128, L], F32R, tag="qkt", name=f"qt{i}")
                  for i in range(NM)]
            kt = [qkt_pool.tile([128, L], F32R, tag="qkt", name=f"kt{i}")
                  for i in range(NM)]
            vext = [vext_pool.tile([128, HPC, DV + 1], BF16, name=f"vext{i}",
                                   tag="vext")
                    for i in range(NLT)]

            # --- stage-A pools (weights, biases, x chunks) ---
            astack = ExitStack()
            xt_pool = astack.enter_context(tc.tile_pool(name="xt", bufs=16))
            w_pool = astack.enter_context(tc.tile_pool(name="w", bufs=NI))
            bias_pool = astack.enter_context(
                tc.tile_pool(name="bias", bufs=2 * NM))
            qb_tile = bias_pool.tile([128, NM], F32, tag="b1", name="bqt",
                                     bufs=2)
            nc.sync.dma_start(
                out=qb_tile, in_=bqd[:].rearrange("(m p) -> p m", p=128))
            kb_tile = bias_pool.tile([128, NM], F32, tag="b1", name="bkt",
                                     bufs=2)
            nc.sync.dma_start(
                out=kb_tile, in_=bkd[:].rearrange("(m p) -> p m", p=128))
            qbias = [qb_tile[:, mm_:mm_ + 1] for mm_ in range(NM)]
            kbias = [kb_tile[:, mm_:mm_ + 1] for mm_ in range(NM)]
            vbias = bias_pool.tile([128, OC], F32, tag="bv", bufs=1)
            nc.sync.dma_start(
                out=vbias, in_=bvd[:].unsqueeze(0).to_broadcast((128, OC)))
            onesf = bias_pool.tile([128, HPC], BF16, tag="ones", bufs=1)
            nc.vector.memset(onesf, 1.0)

            # ---- stage A1: V projection ----
            with tc.tile_pool(name="wv", bufs=NI) as wv_pool:
                wv_tiles = []
                xv_list = []
                for i in range(NI):
                    t = wv_pool.tile([128, OC], F32R, tag="wv", name="wvt_t")
                    if i == 0:
                        nc.sync.dma_start(out=t[:, 0:256], in_=wvt[0:128, 0:256])
                        nc.sync.dma_start(out=t[:, 256:512],
                                          in_=wvt[0:128, 256:512])
                    else:
                        nc.sync.dma_start(
                            out=t, in_=wvt[i * 128:(i + 1) * 128, :])
                    wv_tiles.append(t)
                    t = xt_pool.tile([128, 512], F32R, tag="xt", name="xv")
                    if i == 0:
                        nc.gpsimd.dma_start(out=t[:, 0:256], in_=xtv[0:128, 0:256])
                        nc.gpsimd.dma_start(out=t[:, 256:512],
                                            in_=xtv[0:128, 256:512])
                    else:
                        nc.gpsimd.dma_start(
                            out=t, in_=xtv[i * 128:(i + 1) * 128, 0:512])
                    xv_list.append(t)
                xv_chunks = {0: xv_list}
                wq_tiles, wk_tiles = [], []
                for lc in range(4):
                    if lc in xv_chunks:
                        x_tiles = xv_chunks[lc]
                    else:
                        x_tiles = []
                        for i in range(NI):
                            t = xt_pool.tile([128, 512], F32R, tag="xt",
                                             name="xv")
                            nc.sync.dma_start(
                                out=t,
                                in_=xtv[i * 128:(i + 1) * 128,
                                        lc * 512:(lc + 1) * 512])
                            x_tiles.append(t)
                    if lc == 1:
                        for i in range(NI):
                            t = w_pool.tile([128, OC], F32R, tag="wq",
                                            name="wqt_t")
                            nc.sync.dma_start(
                                out=t, in_=wqt[i * 128:(i + 1) * 128, :])
                            wq_tiles.append(t)
                    elif lc == 2:
                        for i in range(NI):
                            t = w_pool.tile([128, OC], F32R, tag="wk",
                                            name="wkt_t")
                            nc.sync.dma_start(
                                out=t, in_=wkt[i * 128:(i + 1) * 128, :])
                            wk_tiles.append(t)
                    for lsp in range(2):   # two l-subtiles share one psum tile
                        ps = st_pool.tile([128, QW], F32, tag="st", name="psv")
                        for sub in range(2):
                            ls = lsp * 2 + sub
                            for i in range(NI):
                                nc.tensor.matmul(
                                    ps[:, sub * 512:(sub + 1) * 512],
                                    lhsT=x_tiles[i][:, ls * 128:(ls + 1) * 128],
                                    rhs=wv_tiles[i],
                                    start=(i == 0), stop=(i == NI - 1))
                        for sub in range(2):
                            lt = lc * 4 + lsp * 2 + sub
                            nc.vector.tensor_add(
                                vext[lt][:, :, 0:DV],
                                ps[:, sub * 512:(sub + 1) * 512].rearrange(
                                    "p (h d) -> p h d", h=HPC),
                                vbias.rearrange("p (h d) -> p h d", h=HPC))
                            nc.vector.tensor_copy(vext[lt][:, :, DV], onesf)

            # ---- stage A2: Q/K projections, all pairs, one pass over xT ----
            for w_tiles, xsrc, dst, biases in (
                (wq_tiles, xtq, qt, qbias),
                (wk_tiles, xtk, kt, kbias),
            ):
                for lc in range(4):
                    x_tiles = []
                    for i in range(NI):
                        t = xt_pool.tile([128, 512], F32R, tag="xt", name="xp")
                        nc.sync.dma_start(
                            out=t,
                            in_=xsrc[i * 128:(i + 1) * 128,
                                     lc * 512:(lc + 1) * 512])
                        x_tiles.append(t)
                    for mp in range(2):
                        ps = st_pool.tile([128, QW], F32, tag="st", name="psp")
                        for sub in range(2):
                            m = mp * 2 + sub
                            for i in range(NI):
                                nc.tensor.matmul(
                                    ps[:, sub * 512:(sub + 1) * 512],
                                    lhsT=w_tiles[i][:, m * 128:(m + 1) * 128],
                                    rhs=x_tiles[i],
                                    start=(i == 0), stop=(i == NI - 1))
                        for sub in range(2):
                            m = mp * 2 + sub
                            nc.vector.tensor_add(
                                dst[m][:, lc * 512:(lc + 1) * 512],
                                ps[:, sub * 512:(sub + 1) * 512],
                                biases[m].to_broadcast((128, 512)))

            astack.close()

            # ---- stage B: attention per pair ----
            attnt = {}
            bstack = ExitStack()
            attnt_pool = bstack.enter_context(tc.tile_pool(name="attnt",
                                                           bufs=NM))
            ex_pool = bstack.enter_context(tc.tile_pool(name="ex", bufs=3))
            oc_pool = bstack.enter_context(tc.tile_pool(name="oc", bufs=2))
            rr_pool = bstack.enter_context(tc.tile_pool(name="rr", bufs=1))
            wc_pool = bstack.enter_context(tc.tile_pool(name="wc", bufs=NM))
            ob_pool = bstack.enter_context(tc.tile_pool(name="ob", bufs=3))
            wc_tiles = []
            for dt in range(NM):
                t = wc_pool.tile([128, D], F32R, tag="wc", name="wct_t")
                nc.sync.dma_start(out=t, in_=wct[dt * 128:(dt + 1) * 128, :])
                wc_tiles.append(t)
            for m in range(NM):
                attnt[m] = attnt_pool.tile([128, L], F32R, tag="attnt",
                                           name=f"attnt{m}")
                for h2 in range(2):
                    h = m * 2 + h2
                    off = h2 * DV
                    for qc in range(2):
                        ot = ot_pool.tile([DV + 1, QW], F32, tag="ot",
                                          name="ot")
                        sts = {}
                        exs = {}

                        def st_step(k):
                            st = st_pool.tile([128, QW], F32, tag="st",
                                              name="st")
                            for j in range(2):
                                nc.tensor.matmul(
                                    st[:, j * 512:(j + 1) * 512],
                                    lhsT=kt[m][off:off + DV,
                                               k * 128:(k + 1) * 128],
                                    rhs=qt[m][off:off + DV,
                                              qc * QW + j * 512:
                                              qc * QW + (j + 1) * 512],
                                    start=True, stop=True)
                            ex = ex_pool.tile([128, QW], BF16, tag="ex",
                                              name="ex")
                            nc.scalar.activation(
                                out=ex, in_=st, func=EXP, scale=0.125)
                            exs[k] = ex

                        def av_step(k):
                            ex = exs.pop(k)
                            for j in range(2):
                                nc.tensor.matmul(
                                    ot[:, j * 512:(j + 1) * 512],
                                    lhsT=vext[k][:, h, :],
                                    rhs=ex[:, j * 512:(j + 1) * 512],
                                    start=(k == 0), stop=(k == NLT - 1))

                        for k in range(NLT):
                            st_step(k)
                            if k >= 2:
                                av_step(k - 2)
                        av_step(NLT - 2)
                        av_step(NLT - 1)

                        # normalization: stage ot out of PSUM, then scale
                        ocs = oc_pool.tile([DV + 1, QW], F32, tag="oc",
                                           name="ocs")
                        nc.vector.tensor_copy(ocs, ot)
                        rc = rr_pool.tile([1, QW], F32, tag="rc", name="rc")
                        nc.vector.reciprocal(rc, ocs[DV:DV + 1, :])
                        rcd = rcd_pool.tile([QW], F32, tag="rcd", name="rcd")
                        nc.sync.dma_start(out=rcd[:].unsqueeze(0), in_=rc)
                        rb = rr_pool.tile([DV, QW], F32, tag="rb", name="rb")
                        nc.gpsimd.dma_start(
                            out=rb,
                            in_=rcd[:].unsqueeze(0).to_broadcast((DV, QW)))
                        nc.vector.tensor_mul(
                            attnt[m][off:off + DV, qc * QW:(qc + 1) * QW],
                            ocs[0:DV, :],
                            rb)

            # ---- stage C: output projection ----
            for lt in range(NLT):
                ps = st_pool.tile([128, QW], F32, tag="st", name="pso")
                for nck in range(2):
                    for dt in range(NM):
                        nc.tensor.matmul(
                            ps[:, nck * 512:(nck + 1) * 512],
                            lhsT=attnt[dt][:, lt * 128:(lt + 1) * 128],
                            rhs=wc_tiles[dt][:, nck * 512:(nck + 1) * 512],
                            start=(dt == 0), stop=(dt == NM - 1))
                ob = ob_pool.tile([128, QW], F32, tag="ob", name="ob")
                nc.vector.tensor_copy(ob, ps)
                nc.sync.dma_start(
                    out=out[lt * 128:(lt + 1) * 128, :], in_=ob)
            bstack.close()

    nc.compile()
    return nc


def _get_nc():
    if "nc" not in _CACHE:
        _CACHE["nc"] = _build()
    return _CACHE["nc"]


def kernel(query, key, value, Wq, bq, Wk, bk, Wv, bv, Wc, bc, **_unused):
    query = np.asarray(query, np.float32)
    key = np.asarray(key, np.float32)
    value = np.asarray(value, np.float32)
    Wq = np.asarray(Wq, np.float32)
    Wk = np.asarray(Wk, np.float32)
    Wv = np.asarray(Wv, np.float32)
    Wc = np.asarray(Wc, np.float32)
    bq = np.asarray(bq, np.float32)
    bk = np.asarray(bk, np.float32)
    bv = np.asarray(bv, np.float32)
    bc = np.asarray(bc, np.float32)

    nc = _get_nc()

    xtq = [np.ascontiguousarray(query[b].T) for b in range(B)]
    xtk = [np.ascontiguousarray(key[b].T) for b in range(B)]
    xtv = [np.ascontiguousarray(value[b].T) for b in range(B)]
    wqt_g = [np.ascontiguousarray(Wq[g * OC:(g + 1) * OC, :].T) for g in range(2)]
    wkt_g = [np.ascontiguousarray(Wk[g * OC:(g + 1) * OC, :].T) for g in range(2)]
    wvt_g = [np.ascontiguousarray(Wv[g * OC:(g + 1) * OC, :].T) for g in range(2)]
    wct_g = [np.ascontiguousarray(Wc[:, g * OC:(g + 1) * OC].T) for g in range(2)]

    in_maps = []
    for c in range(NCORES):
        b, g = c // 2, c % 2
        in_maps.append({
            "XTQ": xtq[b], "XTK": xtk[b], "XTV": xtv[b],
            "WQT": wqt_g[g], "WKT": wkt_g[g], "WVT": wvt_g[g],
            "WCT": wct_g[g],
            "BQ": np.ascontiguousarray(bq[g * OC:(g + 1) * OC]),
            "BK": np.ascontiguousarray(bk[g * OC:(g + 1) * OC]),
            "BV": np.ascontiguousarray(bv[g * OC:(g + 1) * OC]),
        })

    res = run_bass_kernel_spmd(nc, in_maps, core_ids=list(range(NCORES)),
                               **_CACHE.get("run_kwargs", {}))
    _CACHE["last_results"] = res

    outp = np.empty((B, L, D), np.float32)
    for b in range(B):
        outp[b] = res.results[2 * b]["OUT"] + res.results[2 * b + 1]["OUT"]
    outp += bc
    return outp



# revision 22
# speedup vs baseline: 1.5192x; 1.1076x over previous
"""Multi-head attention forward on 8 Trainium2 NeuronCores (Bass/Tile).

Problem: B=4, L=2048, D=1024, H=16 heads, DV=64.
  out = softmax((x_q Wq^T + bq)(x_k Wk^T + bk)^T / sqrt(DV)) (x_v Wv^T + bv) Wc^T + bc

Sharding (8 cores): core c handles batch b = c//2 and head-group g = c%2
(8 heads = 512 of the 1024 projection columns). Each core produces a
full-shape [L, D] partial of the output projection (contraction over its
512 attention-output dims); the host sums the two partials per batch and
adds bc.

Per-core pipeline (all matmuls fp32r = full-rate fp32 on the PE):
  A. V projection -> natural layout [2048, 8*65] with a ones column per
     head (rowsum trick), then Q/K projections for head-pair 0
     (QT/KT [128, 2048] per pair).
  B. Per head-pair m: attention for its 2 heads. Inner loop per
     (head, q-half): 16 k-tiles; scores^T [k=128, q=1024] in PSUM ->
     ACT exp (scale=1/8, fused) -> SBUF fp32r; AV accumulates [65, 1024]
     in PSUM (ones row = softmax denominator). ACT exp overlaps the PE stream; AV lags 2 k-tiles so all
     semaphore waits are pre-satisfied and the PE stays dense and
     HAM-warm.  Softmax denominator is applied post-AV on
     [64, 1024] tiles (reciprocal + DRAM-roundtrip partition broadcast).
  C. Output projection tail: out[l, n] accumulated over the 4 d-tiles.
"""

from contextlib import ExitStack

import numpy as np
import ml_dtypes

import concourse.bacc as bacc
import concourse.mybir as mybir
from concourse.tile import TileContext
from concourse.bass_utils import run_bass_kernel_spmd

B, L, D, H = 4, 2048, 1024, 16
DV = 64
HPC = 8           # heads per core
OC = HPC * DV     # 512 projection cols per core
NCORES = 8

F32 = mybir.dt.float32
F32R = mybir.dt.float32r
BF16 = mybir.dt.bfloat16
EXP = mybir.ActivationFunctionType.Exp

NI = D // 128    # 8 contraction tiles for projections
NM = OC // 128   # 4 head pairs
NLT = L // 128   # 16 l/k tiles
QW = 1024        # q-half width in stage B

_CACHE = {}


def _build():
    nc = bacc.Bacc("TRN2", target_bir_lowering=False, debug=False,
                   num_devices=NCORES)

    xtq = nc.dram_tensor("XTQ", [D, L], BF16, kind="ExternalInput")
    xtk = nc.dram_tensor("XTK", [D, L], BF16, kind="ExternalInput")
    xtv = nc.dram_tensor("XTV", [D, L], BF16, kind="ExternalInput")
    wqt = nc.dram_tensor("WQT", [D, OC], BF16, kind="ExternalInput")
    wkt = nc.dram_tensor("WKT", [D, OC], BF16, kind="ExternalInput")
    wvt = nc.dram_tensor("WVT", [D, OC], BF16, kind="ExternalInput")
    wct = nc.dram_tensor("WCT", [OC, D], BF16, kind="ExternalInput")
    bqd = nc.dram_tensor("BQ", [OC], F32, kind="ExternalInput")
    bkd = nc.dram_tensor("BK", [OC], F32, kind="ExternalInput")
    bvd = nc.dram_tensor("BV", [OC], F32, kind="ExternalInput")
    out = nc.dram_tensor("OUT", [L, D], F32, kind="ExternalOutput")

    with TileContext(nc) as tc:
        with (
            tc.tile_pool(name="qkt", bufs=2 * NM) as qkt_pool,
            tc.tile_pool(name="vext", bufs=NLT) as vext_pool,
            tc.tile_pool(name="rcd", bufs=2, space="DRAM") as rcd_pool,
            tc.tile_pool(name="st", bufs=3, space="PSUM") as st_pool,
            tc.tile_pool(name="ot", bufs=1, space="PSUM") as ot_pool,
        ):
            qt = [qkt_pool.tile([128, L], BF16, tag="qkt", name=f"qt{i}")
                  for i in range(NM)]
            kt = [qkt_pool.tile([128, L], BF16, tag="qkt", name=f"kt{i}")
                  for i in range(NM)]
            vext = [vext_pool.tile([128, HPC, DV + 1], BF16, name=f"vext{i}",
                                   tag="vext")
                    for i in range(NLT)]

            # --- stage-A pools (weights, biases, x chunks) ---
            astack = ExitStack()
            xt_pool = astack.enter_context(tc.tile_pool(name="xt", bufs=16))
            w_pool = astack.enter_context(tc.tile_pool(name="w", bufs=NI))
            bias_pool = astack.enter_context(
                tc.tile_pool(name="bias", bufs=2 * NM))
            qb_tile = bias_pool.tile([128, NM], F32, tag="b1", name="bqt",
                                     bufs=2)
            nc.sync.dma_start(
                out=qb_tile, in_=bqd[:].rearrange("(m p) -> p m", p=128))
            kb_tile = bias_pool.tile([128, NM], F32, tag="b1", name="bkt",
                                     bufs=2)
            nc.sync.dma_start(
                out=kb_tile, in_=bkd[:].rearrange("(m p) -> p m", p=128))
            qbias = [qb_tile[:, mm_:mm_ + 1] for mm_ in range(NM)]
            kbias = [kb_tile[:, mm_:mm_ + 1] for mm_ in range(NM)]
            vbias = bias_pool.tile([128, OC], F32, tag="bv", bufs=1)
            nc.sync.dma_start(
                out=vbias, in_=bvd[:].unsqueeze(0).to_broadcast((128, OC)))
            onesf = bias_pool.tile([128, HPC], BF16, tag="ones", bufs=1)
            nc.vector.memset(onesf, 1.0)

            # ---- stage A1: V projection ----
            with tc.tile_pool(name="wv", bufs=NI) as wv_pool:
                wv_tiles = []
                xv_list = []
                for i in range(NI):
                    t = wv_pool.tile([128, OC], BF16, tag="wv", name="wvt_t")
                    if i == 0:
                        nc.sync.dma_start(out=t[:, 0:256], in_=wvt[0:128, 0:256])
                        nc.sync.dma_start(out=t[:, 256:512],
                                          in_=wvt[0:128, 256:512])
                    else:
                        nc.sync.dma_start(
                            out=t, in_=wvt[i * 128:(i + 1) * 128, :])
                    wv_tiles.append(t)
                    t = xt_pool.tile([128, 512], BF16, tag="xt", name="xv")
                    if i == 0:
                        nc.gpsimd.dma_start(out=t[:, 0:256], in_=xtv[0:128, 0:256])
                        nc.gpsimd.dma_start(out=t[:, 256:512],
                                            in_=xtv[0:128, 256:512])
                    else:
                        nc.gpsimd.dma_start(
                            out=t, in_=xtv[i * 128:(i + 1) * 128, 0:512])
                    xv_list.append(t)
                xv_chunks = {0: xv_list}
                wq_tiles, wk_tiles = [], []
                for lc in range(4):
                    if lc in xv_chunks:
                        x_tiles = xv_chunks[lc]
                    else:
                        x_tiles = []
                        for i in range(NI):
                            t = xt_pool.tile([128, 512], BF16, tag="xt",
                                             name="xv")
                            nc.sync.dma_start(
                                out=t,
                                in_=xtv[i * 128:(i + 1) * 128,
                                        lc * 512:(lc + 1) * 512])
                            x_tiles.append(t)
                    if lc == 1:
                        for i in range(NI):
                            t = w_pool.tile([128, OC], BF16, tag="wq",
                                            name="wqt_t")
                            nc.sync.dma_start(
                                out=t, in_=wqt[i * 128:(i + 1) * 128, :])
                            wq_tiles.append(t)
                    elif lc == 2:
                        for i in range(NI):
                            t = w_pool.tile([128, OC], BF16, tag="wk",
                                            name="wkt_t")
                            nc.sync.dma_start(
                                out=t, in_=wkt[i * 128:(i + 1) * 128, :])
                            wk_tiles.append(t)
                    for lsp in range(2):   # two l-subtiles share one psum tile
                        ps = st_pool.tile([128, QW], F32, tag="st", name="psv")
                        for sub in range(2):
                            ls = lsp * 2 + sub
                            for i in range(NI):
                                nc.tensor.matmul(
                                    ps[:, sub * 512:(sub + 1) * 512],
                                    lhsT=x_tiles[i][:, ls * 128:(ls + 1) * 128],
                                    rhs=wv_tiles[i],
                                    start=(i == 0), stop=(i == NI - 1))
                        for sub in range(2):
                            lt = lc * 4 + lsp * 2 + sub
                            nc.vector.tensor_add(
                                vext[lt][:, :, 0:DV],
                                ps[:, sub * 512:(sub + 1) * 512].rearrange(
                                    "p (h d) -> p h d", h=HPC),
                                vbias.rearrange("p (h d) -> p h d", h=HPC))
                            nc.vector.tensor_copy(vext[lt][:, :, DV], onesf)

            # ---- stage A2: Q/K projections, all pairs, one pass over xT ----
            for w_tiles, xsrc, dst, biases in (
                (wq_tiles, xtq, qt, qbias),
                (wk_tiles, xtk, kt, kbias),
            ):
                for lc in range(4):
                    x_tiles = []
                    for i in range(NI):
                        t = xt_pool.tile([128, 512], BF16, tag="xt", name="xp")
                        nc.sync.dma_start(
                            out=t,
                            in_=xsrc[i * 128:(i + 1) * 128,
                                     lc * 512:(lc + 1) * 512])
                        x_tiles.append(t)
                    for mp in range(2):
                        ps = st_pool.tile([128, QW], F32, tag="st", name="psp")
                        for sub in range(2):
                            m = mp * 2 + sub
                            for i in range(NI):
                                nc.tensor.matmul(
                                    ps[:, sub * 512:(sub + 1) * 512],
                                    lhsT=w_tiles[i][:, m * 128:(m + 1) * 128],
                                    rhs=x_tiles[i],
                                    start=(i == 0), stop=(i == NI - 1))
                        for sub in range(2):
                            m = mp * 2 + sub
                            nc.vector.tensor_add(
                                dst[m][:, lc * 512:(lc + 1) * 512],
                                ps[:, sub * 512:(sub + 1) * 512],
                                biases[m].to_broadcast((128, 512)))

            astack.close()

            # ---- stage B: attention per pair ----
            attnt = {}
            bstack = ExitStack()
            attnt_pool = bstack.enter_context(tc.tile_pool(name="attnt",
                                                           bufs=NM))
            ex_pool = bstack.enter_context(tc.tile_pool(name="ex", bufs=3))
            oc_pool = bstack.enter_context(tc.tile_pool(name="oc", bufs=2))
            rr_pool = bstack.enter_context(tc.tile_pool(name="rr", bufs=1))
            wc_pool = bstack.enter_context(tc.tile_pool(name="wc", bufs=NM))
            ob_pool = bstack.enter_context(tc.tile_pool(name="ob", bufs=3))
            wc_tiles = []
            for dt in range(NM):
                t = wc_pool.tile([128, D], BF16, tag="wc", name="wct_t")
                nc.sync.dma_start(out=t, in_=wct[dt * 128:(dt + 1) * 128, :])
                wc_tiles.append(t)
            for m in range(NM):
                attnt[m] = attnt_pool.tile([128, L], BF16, tag="attnt",
                                           name=f"attnt{m}")
                for h2 in range(2):
                    h = m * 2 + h2
                    off = h2 * DV
                    for qc in range(2):
                        ot = ot_pool.tile([DV + 1, QW], F32, tag="ot",
                                          name="ot")
                        sts = {}
                        exs = {}

                        def st_step(k):
                            st = st_pool.tile([128, QW], F32, tag="st",
                                              name="st")
                            for j in range(2):
                                nc.tensor.matmul(
                                    st[:, j * 512:(j + 1) * 512],
                                    lhsT=kt[m][off:off + DV,
                                               k * 128:(k + 1) * 128],
                                    rhs=qt[m][off:off + DV,
                                              qc * QW + j * 512:
                                              qc * QW + (j + 1) * 512],
                                    start=True, stop=True)
                            ex = ex_pool.tile([128, QW], BF16, tag="ex",
                                              name="ex")
                            nc.scalar.activation(
                                out=ex, in_=st, func=EXP, scale=0.125)
                            exs[k] = ex

                        def av_step(k):
                            ex = exs.pop(k)
                            for j in range(2):
                                nc.tensor.matmul(
                                    ot[:, j * 512:(j + 1) * 512],
                                    lhsT=vext[k][:, h, :],
                                    rhs=ex[:, j * 512:(j + 1) * 512],
                                    start=(k == 0), stop=(k == NLT - 1))

                        for k in range(NLT):
                            st_step(k)
                            if k >= 2:
                                av_step(k - 2)
                        av_step(NLT - 2)
                        av_step(NLT - 1)

                        # normalization: stage ot out of PSUM, then scale
                        ocs = oc_pool.tile([DV + 1, QW], F32, tag="oc",
                                           name="ocs")
                        nc.vector.tensor_copy(ocs, ot)
                        rc = rr_pool.tile([1, QW], F32, tag="rc", name="rc")
                        nc.vector.reciprocal(rc, ocs[DV:DV + 1, :])
                        rcd = rcd_pool.tile([QW], F32, tag="rcd", name="rcd")
                        nc.sync.dma_start(out=rcd[:].unsqueeze(0), in_=rc)
                        rb = rr_pool.tile([DV, QW], F32, tag="rb", name="rb")
                        nc.gpsimd.dma_start(
                            out=rb,
                            in_=rcd[:].unsqueeze(0).to_broadcast((DV, QW)))
                        nc.vector.tensor_mul(
                            attnt[m][off:off + DV, qc * QW:(qc + 1) * QW],
                            ocs[0:DV, :],
                            rb)

            # ---- stage C: output projection ----
            for lt in range(NLT):
                ps = st_pool.tile([128, QW], F32, tag="st", name="pso")
                for nck in range(2):
                    for dt in range(NM):
                        nc.tensor.matmul(
                            ps[:, nck * 512:(nck + 1) * 512],
                            lhsT=attnt[dt][:, lt * 128:(lt + 1) * 128],
                            rhs=wc_tiles[dt][:, nck * 512:(nck + 1) * 512],
                            start=(dt == 0), stop=(dt == NM - 1))
                ob = ob_pool.tile([128, QW], F32, tag="ob", name="ob")
                nc.scalar.copy(ob, ps)
                nc.sync.dma_start(
                    out=out[lt * 128:(lt + 1) * 128, :], in_=ob)
            bstack.close()

    nc.compile()
    return nc


def _get_nc():
    if "nc" not in _CACHE:
        _CACHE["nc"] = _build()
    return _CACHE["nc"]


def kernel(query, key, value, Wq, bq, Wk, bk, Wv, bv, Wc, bc, **_unused):
    query = np.asarray(query, np.float32)
    key = np.asarray(key, np.float32)
    value = np.asarray(value, np.float32)
    Wq = np.asarray(Wq, np.float32)
    Wk = np.asarray(Wk, np.float32)
    Wv = np.asarray(Wv, np.float32)
    Wc = np.asarray(Wc, np.float32)
    bq = np.asarray(bq, np.float32)
    bk = np.asarray(bk, np.float32)
    bv = np.asarray(bv, np.float32)
    bc = np.asarray(bc, np.float32)

    nc = _get_nc()

    xtq = [np.ascontiguousarray(query[b].T.astype(ml_dtypes.bfloat16)) for b in range(B)]
    xtk = [np.ascontiguousarray(key[b].T.astype(ml_dtypes.bfloat16)) for b in range(B)]
    xtv = [np.ascontiguousarray(value[b].T.astype(ml_dtypes.bfloat16)) for b in range(B)]
    wqt_g = [np.ascontiguousarray(Wq[g * OC:(g + 1) * OC, :].T.astype(ml_dtypes.bfloat16)) for g in range(2)]
    wkt_g = [np.ascontiguousarray(Wk[g * OC:(g + 1) * OC, :].T.astype(ml_dtypes.bfloat16)) for g in range(2)]
    wvt_g = [np.ascontiguousarray(Wv[g * OC:(g + 1) * OC, :].T.astype(ml_dtypes.bfloat16)) for g in range(2)]
    wct_g = [np.ascontiguousarray(Wc[:, g * OC:(g + 1) * OC].T.astype(ml_dtypes.bfloat16)) for g in range(2)]

    in_maps = []
    for c in range(NCORES):
        b, g = c // 2, c % 2
        in_maps.append({
            "XTQ": xtq[b], "XTK": xtk[b], "XTV": xtv[b],
            "WQT": wqt_g[g], "WKT": wkt_g[g], "WVT": wvt_g[g],
            "WCT": wct_g[g],
            "BQ": np.ascontiguousarray(bq[g * OC:(g + 1) * OC]),
            "BK": np.ascontiguousarray(bk[g * OC:(g + 1) * OC]),
            "BV": np.ascontiguousarray(bv[g * OC:(g + 1) * OC]),
        })

    res = run_bass_kernel_spmd(nc, in_maps, core_ids=list(range(NCORES)),
                               **_CACHE.get("run_kwargs", {}))
    _CACHE["last_results"] = res

    outp = np.empty((B, L, D), np.float32)
    for b in range(B):
        outp[b] = res.results[2 * b]["OUT"] + res.results[2 * b + 1]["OUT"]
    outp += bc
    return outp



# revision 23
# speedup vs baseline: 1.5314x; 1.0080x over previous
"""Multi-head attention forward on 8 Trainium2 NeuronCores (Bass/Tile).

Problem: B=4, L=2048, D=1024, H=16 heads, DV=64.
  out = softmax((x_q Wq^T + bq)(x_k Wk^T + bk)^T / sqrt(DV)) (x_v Wv^T + bv) Wc^T + bc

Sharding (8 cores): core c handles batch b = c//2 and head-group g = c%2
(8 heads = 512 of the 1024 projection columns). Each core produces a
full-shape [L, D] partial of the output projection (contraction over its
512 attention-output dims); the host sums the two partials per batch and
adds bc.

Per-core pipeline (all matmuls fp32r = full-rate fp32 on the PE):
  A. V projection -> natural layout [2048, 8*65] with a ones column per
     head (rowsum trick), then Q/K projections for head-pair 0
     (QT/KT [128, 2048] per pair).
  B. Per head-pair m: attention for its 2 heads. Inner loop per
     (head, q-half): 16 k-tiles; scores^T [k=128, q=1024] in PSUM ->
     ACT exp (scale=1/8, fused) -> SBUF fp32r; AV accumulates [65, 1024]
     in PSUM (ones row = softmax denominator). ACT exp overlaps the PE stream; AV lags 2 k-tiles so all
     semaphore waits are pre-satisfied and the PE stays dense and
     HAM-warm.  Softmax denominator is applied post-AV on
     [64, 1024] tiles (reciprocal + DRAM-roundtrip partition broadcast).
  C. Output projection tail: out[l, n] accumulated over the 4 d-tiles.
"""

from contextlib import ExitStack

import numpy as np
import ml_dtypes

import concourse.bacc as bacc
import concourse.mybir as mybir
from concourse.tile import TileContext
from concourse.bass_utils import run_bass_kernel_spmd

B, L, D, H = 4, 2048, 1024, 16
DV = 64
HPC = 8           # heads per core
OC = HPC * DV     # 512 projection cols per core
NCORES = 8

F32 = mybir.dt.float32
F32R = mybir.dt.float32r
BF16 = mybir.dt.bfloat16
EXP = mybir.ActivationFunctionType.Exp

NI = D // 128    # 8 contraction tiles for projections
NM = OC // 128   # 4 head pairs
NLT = L // 128   # 16 l/k tiles
QW = 1024        # q-half width in stage B

_CACHE = {}


def _build():
    nc = bacc.Bacc("TRN2", target_bir_lowering=False, debug=False,
                   num_devices=NCORES)

    xtq = nc.dram_tensor("XTQ", [D, L], BF16, kind="ExternalInput")
    xtk = nc.dram_tensor("XTK", [D, L], BF16, kind="ExternalInput")
    xtv = nc.dram_tensor("XTV", [D, L], BF16, kind="ExternalInput")
    wqt = nc.dram_tensor("WQT", [D, OC], BF16, kind="ExternalInput")
    wkt = nc.dram_tensor("WKT", [D, OC], BF16, kind="ExternalInput")
    wvt = nc.dram_tensor("WVT", [D, OC], BF16, kind="ExternalInput")
    wct = nc.dram_tensor("WCT", [OC, D], BF16, kind="ExternalInput")
    bqd = nc.dram_tensor("BQ", [OC], F32, kind="ExternalInput")
    bkd = nc.dram_tensor("BK", [OC], F32, kind="ExternalInput")
    bvd = nc.dram_tensor("BV", [OC], F32, kind="ExternalInput")
    out = nc.dram_tensor("OUT", [L, D], F32, kind="ExternalOutput")

    with TileContext(nc) as tc:
        with (
            tc.tile_pool(name="qkt", bufs=2 * NM) as qkt_pool,
            tc.tile_pool(name="vext", bufs=NLT) as vext_pool,
            tc.tile_pool(name="rcd", bufs=2, space="DRAM") as rcd_pool,
            tc.tile_pool(name="st", bufs=2, space="PSUM") as st_pool,
            tc.tile_pool(name="ot", bufs=2, space="PSUM") as ot_pool,
        ):
            qt = [qkt_pool.tile([128, L], BF16, tag="qkt", name=f"qt{i}")
                  for i in range(NM)]
            kt = [qkt_pool.tile([128, L], BF16, tag="qkt", name=f"kt{i}")
                  for i in range(NM)]
            vext = [vext_pool.tile([128, HPC, DV + 1], BF16, name=f"vext{i}",
                                   tag="vext")
                    for i in range(NLT)]

            # --- stage-A pools (weights, biases, x chunks) ---
            astack = ExitStack()
            xt_pool = astack.enter_context(tc.tile_pool(name="xt", bufs=16))
            w_pool = astack.enter_context(tc.tile_pool(name="w", bufs=NI))
            bias_pool = astack.enter_context(
                tc.tile_pool(name="bias", bufs=2 * NM))
            qb_tile = bias_pool.tile([128, NM], F32, tag="b1", name="bqt",
                                     bufs=2)
            nc.sync.dma_start(
                out=qb_tile, in_=bqd[:].rearrange("(m p) -> p m", p=128))
            kb_tile = bias_pool.tile([128, NM], F32, tag="b1", name="bkt",
                                     bufs=2)
            nc.sync.dma_start(
                out=kb_tile, in_=bkd[:].rearrange("(m p) -> p m", p=128))
            qbias = [qb_tile[:, mm_:mm_ + 1] for mm_ in range(NM)]
            kbias = [kb_tile[:, mm_:mm_ + 1] for mm_ in range(NM)]
            vbias = bias_pool.tile([128, OC], F32, tag="bv", bufs=1)
            nc.sync.dma_start(
                out=vbias, in_=bvd[:].unsqueeze(0).to_broadcast((128, OC)))
            onesf = bias_pool.tile([128, HPC], BF16, tag="ones", bufs=1)
            nc.vector.memset(onesf, 1.0)

            # ---- stage A1: V projection ----
            with tc.tile_pool(name="wv", bufs=NI) as wv_pool:
                wv_tiles = []
                xv_list = []
                for i in range(NI):
                    t = wv_pool.tile([128, OC], BF16, tag="wv", name="wvt_t")
                    if i == 0:
                        nc.sync.dma_start(out=t[:, 0:256], in_=wvt[0:128, 0:256])
                        nc.sync.dma_start(out=t[:, 256:512],
                                          in_=wvt[0:128, 256:512])
                    else:
                        nc.sync.dma_start(
                            out=t, in_=wvt[i * 128:(i + 1) * 128, :])
                    wv_tiles.append(t)
                    t = xt_pool.tile([128, 512], BF16, tag="xt", name="xv")
                    if i == 0:
                        nc.gpsimd.dma_start(out=t[:, 0:256], in_=xtv[0:128, 0:256])
                        nc.gpsimd.dma_start(out=t[:, 256:512],
                                            in_=xtv[0:128, 256:512])
                    else:
                        nc.gpsimd.dma_start(
                            out=t, in_=xtv[i * 128:(i + 1) * 128, 0:512])
                    xv_list.append(t)
                xv_chunks = {0: xv_list}
                wq_tiles, wk_tiles = [], []
                for lc in range(4):
                    if lc in xv_chunks:
                        x_tiles = xv_chunks[lc]
                    else:
                        x_tiles = []
                        for i in range(NI):
                            t = xt_pool.tile([128, 512], BF16, tag="xt",
                                             name="xv")
                            nc.sync.dma_start(
                                out=t,
                                in_=xtv[i * 128:(i + 1) * 128,
                                        lc * 512:(lc + 1) * 512])
                            x_tiles.append(t)
                    if lc == 1:
                        for i in range(NI):
                            t = w_pool.tile([128, OC], BF16, tag="wq",
                                            name="wqt_t")
                            nc.sync.dma_start(
                                out=t, in_=wqt[i * 128:(i + 1) * 128, :])
                            wq_tiles.append(t)
                    elif lc == 2:
                        for i in range(NI):
                            t = w_pool.tile([128, OC], BF16, tag="wk",
                                            name="wkt_t")
                            nc.sync.dma_start(
                                out=t, in_=wkt[i * 128:(i + 1) * 128, :])
                            wk_tiles.append(t)
                    for lsp in range(2):   # two l-subtiles share one psum tile
                        ps = st_pool.tile([128, QW], F32, tag="st", name="psv")
                        for sub in range(2):
                            ls = lsp * 2 + sub
                            for i in range(NI):
                                nc.tensor.matmul(
                                    ps[:, sub * 512:(sub + 1) * 512],
                                    lhsT=x_tiles[i][:, ls * 128:(ls + 1) * 128],
                                    rhs=wv_tiles[i],
                                    start=(i == 0), stop=(i == NI - 1))
                        for sub in range(2):
                            lt = lc * 4 + lsp * 2 + sub
                            nc.vector.tensor_add(
                                vext[lt][:, :, 0:DV],
                                ps[:, sub * 512:(sub + 1) * 512].rearrange(
                                    "p (h d) -> p h d", h=HPC),
                                vbias.rearrange("p (h d) -> p h d", h=HPC))
                            nc.vector.tensor_copy(vext[lt][:, :, DV], onesf)

            # ---- stage A2: Q/K projections, all pairs, one pass over xT ----
            for w_tiles, xsrc, dst, biases in (
                (wq_tiles, xtq, qt, qbias),
                (wk_tiles, xtk, kt, kbias),
            ):
                for lc in range(4):
                    x_tiles = []
                    for i in range(NI):
                        t = xt_pool.tile([128, 512], BF16, tag="xt", name="xp")
                        nc.sync.dma_start(
                            out=t,
                            in_=xsrc[i * 128:(i + 1) * 128,
                                     lc * 512:(lc + 1) * 512])
                        x_tiles.append(t)
                    for mp in range(2):
                        ps = st_pool.tile([128, QW], F32, tag="st", name="psp")
                        for sub in range(2):
                            m = mp * 2 + sub
                            for i in range(NI):
                                nc.tensor.matmul(
                                    ps[:, sub * 512:(sub + 1) * 512],
                                    lhsT=w_tiles[i][:, m * 128:(m + 1) * 128],
                                    rhs=x_tiles[i],
                                    start=(i == 0), stop=(i == NI - 1))
                        for sub in range(2):
                            m = mp * 2 + sub
                            nc.vector.tensor_add(
                                dst[m][:, lc * 512:(lc + 1) * 512],
                                ps[:, sub * 512:(sub + 1) * 512],
                                biases[m].to_broadcast((128, 512)))

            astack.close()

            # ---- stage B: attention per pair ----
            attnt = {}
            bstack = ExitStack()
            attnt_pool = bstack.enter_context(tc.tile_pool(name="attnt",
                                                           bufs=NM))
            ex_pool = bstack.enter_context(tc.tile_pool(name="ex", bufs=6))
            oc_pool = bstack.enter_context(tc.tile_pool(name="oc", bufs=2))
            rr_pool = bstack.enter_context(tc.tile_pool(name="rr", bufs=1))
            wc_pool = bstack.enter_context(tc.tile_pool(name="wc", bufs=NM))
            ob_pool = bstack.enter_context(tc.tile_pool(name="ob", bufs=3))
            wc_tiles = []
            for dt in range(NM):
                t = wc_pool.tile([128, D], BF16, tag="wc", name="wct_t")
                nc.sync.dma_start(out=t, in_=wct[dt * 128:(dt + 1) * 128, :])
                wc_tiles.append(t)
            for m in range(NM):
                attnt[m] = attnt_pool.tile([128, L], BF16, tag="attnt",
                                           name=f"attnt{m}")
                for h2 in range(2):
                    h = m * 2 + h2
                    off = h2 * DV
                    for qc in range(2):
                        ot = ot_pool.tile([DV + 1, QW], F32, tag="ot",
                                          name="ot")
                        sts = {}
                        exs = {}

                        def st_step(k):
                            st = st_pool.tile([128, QW], F32, tag="st",
                                              name="st")
                            for j in range(2):
                                nc.tensor.matmul(
                                    st[:, j * 512:(j + 1) * 512],
                                    lhsT=kt[m][off:off + DV,
                                               k * 128:(k + 1) * 128],
                                    rhs=qt[m][off:off + DV,
                                              qc * QW + j * 512:
                                              qc * QW + (j + 1) * 512],
                                    start=True, stop=True)
                            ex = ex_pool.tile([128, QW], BF16, tag="ex",
                                              name="ex")
                            nc.scalar.activation(
                                out=ex, in_=st, func=EXP, scale=0.125)
                            exs[k] = ex

                        def av_step(k):
                            ex = exs.pop(k)
                            for j in range(2):
                                nc.tensor.matmul(
                                    ot[:, j * 512:(j + 1) * 512],
                                    lhsT=vext[k][:, h, :],
                                    rhs=ex[:, j * 512:(j + 1) * 512],
                                    start=(k == 0), stop=(k == NLT - 1))

                        for k in range(NLT):
                            st_step(k)
                            if k >= 2:
                                av_step(k - 2)
                        av_step(NLT - 2)
                        av_step(NLT - 1)

                        # normalization: stage ot out of PSUM, then scale
                        ocs = oc_pool.tile([DV + 1, QW], F32, tag="oc",
                                           name="ocs")
                        nc.vector.tensor_copy(ocs, ot)
                        rc = rr_pool.tile([1, QW], F32, tag="rc", name="rc")
                        nc.vector.reciprocal(rc, ocs[DV:DV + 1, :])
                        rcd = rcd_pool.tile([QW], F32, tag="rcd", name="rcd")
                        nc.sync.dma_start(out=rcd[:].unsqueeze(0), in_=rc)
                        rb = rr_pool.tile([DV, QW], F32, tag="rb", name="rb")
                        nc.gpsimd.dma_start(
                            out=rb,
                            in_=rcd[:].unsqueeze(0).to_broadcast((DV, QW)))
                        nc.vector.tensor_mul(
                            attnt[m][off:off + DV, qc * QW:(qc + 1) * QW],
                            ocs[0:DV, :],
                            rb)

            # ---- stage C: output projection ----
            for lt in range(NLT):
                ps = st_pool.tile([128, QW], F32, tag="st", name="pso")
                for nck in range(2):
                    for dt in range(NM):
                        nc.tensor.matmul(
                            ps[:, nck * 512:(nck + 1) * 512],
                            lhsT=attnt[dt][:, lt * 128:(lt + 1) * 128],
                            rhs=wc_tiles[dt][:, nck * 512:(nck + 1) * 512],
                            start=(dt == 0), stop=(dt == NM - 1))
                ob = ob_pool.tile([128, QW], F32, tag="ob", name="ob")
                nc.scalar.copy(ob, ps)
                nc.sync.dma_start(
                    out=out[lt * 128:(lt + 1) * 128, :], in_=ob)
            bstack.close()

    nc.compile()
    return nc


def _get_nc():
    if "nc" not in _CACHE:
        _CACHE["nc"] = _build()
    return _CACHE["nc"]


def kernel(query, key, value, Wq, bq, Wk, bk, Wv, bv, Wc, bc, **_unused):
    query = np.asarray(query, np.float32)
    key = np.asarray(key, np.float32)
    value = np.asarray(value, np.float32)
    Wq = np.asarray(Wq, np.float32)
    Wk = np.asarray(Wk, np.float32)
    Wv = np.asarray(Wv, np.float32)
    Wc = np.asarray(Wc, np.float32)
    bq = np.asarray(bq, np.float32)
    bk = np.asarray(bk, np.float32)
    bv = np.asarray(bv, np.float32)
    bc = np.asarray(bc, np.float32)

    nc = _get_nc()

    xtq = [np.ascontiguousarray(query[b].T.astype(ml_dtypes.bfloat16)) for b in range(B)]
    xtk = [np.ascontiguousarray(key[b].T.astype(ml_dtypes.bfloat16)) for b in range(B)]
    xtv = [np.ascontiguousarray(value[b].T.astype(ml_dtypes.bfloat16)) for b in range(B)]
    wqt_g = [np.ascontiguousarray(Wq[g * OC:(g + 1) * OC, :].T.astype(ml_dtypes.bfloat16)) for g in range(2)]
    wkt_g = [np.ascontiguousarray(Wk[g * OC:(g + 1) * OC, :].T.astype(ml_dtypes.bfloat16)) for g in range(2)]
    wvt_g = [np.ascontiguousarray(Wv[g * OC:(g + 1) * OC, :].T.astype(ml_dtypes.bfloat16)) for g in range(2)]
    wct_g = [np.ascontiguousarray(Wc[:, g * OC:(g + 1) * OC].T.astype(ml_dtypes.bfloat16)) for g in range(2)]

    in_maps = []
    for c in range(NCORES):
        b, g = c // 2, c % 2
        in_maps.append({
            "XTQ": xtq[b], "XTK": xtk[b], "XTV": xtv[b],
            "WQT": wqt_g[g], "WKT": wkt_g[g], "WVT": wvt_g[g],
            "WCT": wct_g[g],
            "BQ": np.ascontiguousarray(bq[g * OC:(g + 1) * OC]),
            "BK": np.ascontiguousarray(bk[g * OC:(g + 1) * OC]),
            "BV": np.ascontiguousarray(bv[g * OC:(g + 1) * OC]),
        })

    res = run_bass_kernel_spmd(nc, in_maps, core_ids=list(range(NCORES)),
                               **_CACHE.get("run_kwargs", {}))
    _CACHE["last_results"] = res

    outp = np.empty((B, L, D), np.float32)
    for b in range(B):
        outp[b] = res.results[2 * b]["OUT"] + res.results[2 * b + 1]["OUT"]
    outp += bc
    return outp

